# revision 15
# baseline (speedup 1.0000x reference)
"""v3: per-partition gather offsets (HW-validated semantics).

- topk on DVE (bf16 by default, f32 fallback): 4 rounds max/max_index/match_replace
- stream-transpose [32,32] puts idx one-per-partition: tout[j,b] = rank-j channel of row b
- 4 per-row indirect DMAs: offsets tout[3:25, b], element_offset = b*C*HW
- reduce: DVE low cols + Act high cols, combined, PE matmul vs ones/denom
"""

import numpy as np

import concourse.bass as bass
import concourse.mybir as mybir
from concourse.bass_utils import run_bass_kernel_spmd

B, C, H, W = 32, 1000, 56, 56
HW = H * W  # 3136
NCORES = 8
BL = B // NCORES  # 4
TOPK, SKIP = 25, 3
SEL = TOPK - SKIP  # 22
NGAT = SEL * BL  # 88
NP = NGAT + BL  # 92
DENOM = float(B * HW)
NEG = -3.0e38
SPLIT = 1600


def build_nc(guard=True) -> bass.Bass:
    nc = bass.Bass(detect_race_conditions=guard)
    mdt = mybir.dt.float32

    feat = nc.declare_dram_parameter(
        "features", [BL * C, HW], mybir.dt.float32, isOutput=False
    )
    mo = nc.declare_dram_parameter("main_out", [BL, C], mdt, isOutput=False)
    p_in = nc.declare_dram_parameter("p", [BL, HW], mybir.dt.float32, isOutput=False)
    out_ext = nc.declare_dram_parameter("out", [1, 1], mybir.dt.float32, isOutput=True)

    from contextlib import ExitStack

    with ExitStack() as ctx:
        e = ctx.enter_context
        m0 = e(nc.sbuf_tensor([BL, C], mdt))
        m1 = e(nc.sbuf_tensor([BL, C], mdt))
        m2 = e(nc.sbuf_tensor([BL, C], mdt))
        m3 = e(nc.sbuf_tensor([BL, C], mdt))
        vals = e(nc.sbuf_tensor([BL, 32], mdt))
        idx = e(nc.sbuf_tensor([32, 32], mybir.dt.uint32))
        tout = e(nc.sbuf_tensor([32, 32], mybir.dt.uint32))
        gat = e(nc.sbuf_tensor([NP, HW], mybir.dt.float32))
        dump = e(nc.sbuf_tensor([NP, HW - SPLIT], mybir.dt.float32))
        colsum = e(nc.sbuf_tensor([NP, 2], mybir.dt.float32))
        colf = e(nc.sbuf_tensor([NP, 1], mybir.dt.float32))
        ones = e(nc.sbuf_tensor([NP, 1], mybir.dt.float32))
        res = e(nc.sbuf_tensor([1, 1], mybir.dt.float32))
        warm = e(nc.sbuf_tensor([1, 1], mybir.dt.float32))
        acc = e(nc.psum_tensor([1, 1], mybir.dt.float32))
        s_mo = e(nc.semaphore())
        s_p = e(nc.semaphore())
        s_out = e(nc.semaphore())
        s_gat = e(nc.semaphore())
        s_mm = e(nc.semaphore())
        s_act = e(nc.semaphore())
        s_red = e(nc.semaphore())
        s_dve = e(nc.semaphore())
        block = e(nc.Block())

        marks = {}

        @block.sync
        def _(sync):
            sync.dma_start(m0[:], mo[:]).then_inc(s_mo, 16)
            sync.dma_start(gat[NGAT:NP, :], p_in[:]).then_inc(s_p, 16)
            sync.wait_ge(s_red, 1)
            sync.dma_start(out_ext[:], res[:]).then_inc(s_out, 16)

        @block.vector
        def _(vector):
            n = 0

            def step(emit):
                nonlocal n
                if guard and n:
                    vector.wait_ge(s_dve, n)
                inst = emit()
                inst.then_inc(s_dve, 1)
                n += 1
                return inst

            step(lambda: vector.memset(ones[:], 1.0 / DENOM))
            step(lambda: vector.memset(idx[:], 0))
            vector.wait_ge(s_mo, 16)
            bufs = [m0, m1, m2, m3]
            for r in range(4):
                src = bufs[r]
                step(lambda: vector.max(vals[:, 8 * r : 8 * r + 8], src[:]))
                if r < 3:
                    step(
                        lambda: vector.match_replace(
                            bufs[r + 1][:], vals[:, 8 * r : 8 * r + 8], src[:], NEG
                        )
                    )
            # ranks 3..24 channel ids vs pristine m0, shifted to cols 0..21
            # (the HW indirect-DMA offset AP always reads from partition 0)
            for lo, hi, d in [(3, 11, 0), (11, 19, 8), (17, 25, 14)]:
                step(
                    lambda: vector.max_index(idx[0:BL, d : d + 8], vals[:, lo:hi], m0[:])
                )
            step(lambda: vector.transpose(tout[:], idx[:]))
            marks["tout"] = n
            vector.wait_ge(s_gat, 16 * BL)
            vector.wait_ge(s_p, 16)
            step(
                lambda: vector.reduce_sum(
                    colsum[:, 0:1], gat[:, 0:SPLIT], axis=mybir.AxisListType.X
                )
            )
            vector.wait_ge(s_act, 1)
            step(
                lambda: vector.tensor_tensor(
                    out=colf[:],
                    in0=colsum[:, 0:1],
                    in1=colsum[:, 1:2],
                    op=mybir.AluOpType.add,
                )
            )
            marks["colf"] = n
            vector.wait_ge(s_mm, 1)
            if guard:
                vector.wait_ge(s_dve, n)
            vector.tensor_copy(res[:], acc[:]).then_inc(s_red, 1)

        @block.scalar
        def _(scalar):
            # warm-up so any act-table load happens off the critical path
            scalar.wait_ge(s_dve, 1)
            scalar.activation(warm[:], ones[0:1, :], mybir.ActivationFunctionType.Copy)
            scalar.wait_ge(s_gat, 16 * BL)
            scalar.wait_ge(s_p, 16)
            scalar.activation(
                dump[:],
                gat[:, SPLIT:HW],
                mybir.ActivationFunctionType.Copy,
                accum_out=colsum[:, 1:2],
            ).then_inc(s_act, 1)

        @block.gpsimd
        def _(gpsimd):
            gpsimd.wait_ge(s_dve, marks["tout"])
            for b in range(BL):
                gpsimd.indirect_dma_start(
                    out=gat[b * SEL : (b + 1) * SEL, :],
                    out_offset=None,
                    in_=feat[:],
                    in_offset=bass.IndirectOffsetOnAxis(
                        ap=tout[0:SEL, b : b + 1], axis=0
                    ),
                    element_offset=b * C * HW,
                ).then_inc(s_gat, 16)

        @block.tensor
        def _(tensor):
            tensor.wait_ge(s_dve, marks["colf"])
            tensor.matmul(acc[:], ones[:], colf[:]).then_inc(s_mm, 1)

    return nc


def _to_bf16(x: np.ndarray) -> np.ndarray:
    import ml_dtypes

    return x.astype(ml_dtypes.bfloat16)


def shard_inputs(p, main_out, features):
    mo = main_out
    in_maps = []
    for i in range(NCORES):
        sl = slice(i * BL, (i + 1) * BL)
        in_maps.append(
            {
                "features": features[sl].reshape(BL * C, HW),
                "main_out": mo[sl],
                "p": p[sl].reshape(BL, HW),
            }
        )
    return in_maps


def kernel(p, main_out, features, return_res=False, guard=True):
    p = np.ascontiguousarray(np.asarray(p, dtype=np.float32))
    main_out = np.ascontiguousarray(np.asarray(main_out, dtype=np.float32))
    features = np.ascontiguousarray(np.asarray(features, dtype=np.float32))

    nc = build_nc(guard=guard)
    in_maps = shard_inputs(p, main_out, features)
    res = run_bass_kernel_spmd(nc, in_maps, core_ids=list(range(NCORES)))
    total = np.float32(0.0)
    for r in res.results:
        total += r["out"][0, 0]
    out = np.asarray(total, dtype=np.float32)
    if return_res:
        return out, res
    return out
